# revision 57
# baseline (speedup 1.0000x reference)
"""Explorer GNN message-passing kernel for 8 TRN2 NeuronCores (Bass/Tile).

Strategy (node-sharded, edge-local), v2:
  - Nodes split contiguously across 8 cores. Each core owns every edge whose
    dst falls in its range, so segment-max is core-local. Owned nodes are
    permuted by ascending in-degree into "slots" (blocks of 128); edges laid
    out in (round, block, partition) order so one round-tile of messages
    max-combines into a contiguous column range of the feature-major node
    state with a single DVE max.
  - Node state is kept as x~ = x - fx_b2 on SBUF partitions 64:128 for the
    whole kernel; all biases are folded so scatter-max and the y-update need
    no bias ops at all.
  - x rows are exchanged between cores as bf16 256-byte rows; the per-edge
    x[src] gather uses dma_gather(transpose=True), which lands the rows
    FEATURE-MAJOR in SBUF - no PE transposes in the inner loop, and all
    inner-loop matmuls run in bf16 (2x PE rate):
      z1  = [fyB|fxA]^T xj~  (+)  [fyA|fxB]^T xi~   (two K=64->M=128 passes)
      z1x += fxC^T y~                                (K=64 quadrant pass)
      z2  = diag(fy_w2, fx_w2)^T [h1y(t); h1x(t-1)]  (one K=128->M=128 pass,
            software-pipelined across tiles)
  - The edge-init MLP (hy) is similarly pipelined into ONE matmul per tile
    with lhsT [[0,hyAB],[hy_w2,0]].
"""

import os
import sys
import numpy as np
import ml_dtypes

import concourse.bass as bass
import concourse.mybir as mybir
import concourse.bacc as bacc
import concourse.tile as tile
from concourse.bass_utils import run_bass_kernel_spmd
from concourse.masks import make_identity

NCORE = 8
P = 128
H = 64
TILE_W = 512
GC = 1024   # max indices per dma_gather call (ring limit < 2048)
# NOTE: dma_gather(transpose=True) silently corrupts data beyond ~16
# back-to-back calls (HW-verified); gathered rows are transposed on the
# PE instead (bf16: 1 cyc/row).
NEG = -1.0e30
F32 = mybir.dt.float32
BF16 = mybir.dt.bfloat16
I16 = mybir.dt.int16
NPBF16 = ml_dtypes.bfloat16

LAST_EXEC_NS = None
LAST_TRACE = None
_BUILD_CACHE = {}
SIM_SINGLE = False  # build single-core variant (collective -> DMA)


def _log(msg):
    print(f"[kernel] {msg}", file=sys.stderr, flush=True)


def _wrap16(ids, ncols):
    out = np.zeros((16, ncols), dtype=np.int16)
    n = len(ids)
    out[np.arange(n) % 16, np.arange(n) // 16] = ids.astype(np.int16)
    return out


def _wrap16_chunks(ids, chunk):
    n = len(ids)
    out = np.zeros((16, n // 16), dtype=np.int16)
    off = 0
    while off < n:
        m = min(chunk, n - off)
        out[:, off // 16:(off + m) // 16] = _wrap16(ids[off:off + m], m // 16)
        off += m
    return out


def _preprocess(v, labels, edge_index):
    N, C = v.shape
    D = C + 2
    E = edge_index.shape[1]
    NODE_LOC = (N + NCORE - 1) // NCORE
    NBLK = (NODE_LOC + P - 1) // P
    S_NODE = NBLK * P

    vc = np.concatenate([v, labels], axis=1).astype(np.float32)  # [N, D]
    gi = int(np.argmax(labels[:, 1]))
    goal = vc[gi]
    d = vc - goal
    feat36 = np.concatenate(
        [vc, np.broadcast_to(goal, vc.shape), d, d * d], axis=1
    ).astype(np.float32)  # [N, 4D]

    src = edge_index[0].astype(np.int64)
    dst = edge_index[1].astype(np.int64)
    owner = dst // NODE_LOC

    cores = []
    for c in range(NCORE):
        lo, hi = c * NODE_LOC, min((c + 1) * NODE_LOC, N)
        nloc = hi - lo
        eids = np.where(owner == c)[0]
        dl = dst[eids] - lo
        deg = np.bincount(dl, minlength=nloc)
        order = np.argsort(deg, kind="stable")  # ascending degree
        slot_of_local = np.empty(nloc, dtype=np.int64)
        slot_of_local[order] = np.arange(nloc)
        es = eids[np.argsort(dl, kind="stable")]
        rp = np.zeros(nloc + 1, dtype=np.int64)
        rp[1:] = np.cumsum(deg)
        deg_s = np.zeros(S_NODE, dtype=np.int64)
        deg_s[:nloc] = deg[order]
        node_s = np.full(S_NODE, -1, dtype=np.int64)
        node_s[:nloc] = order + lo
        rp_s = np.zeros(S_NODE, dtype=np.int64)
        rp_s[:nloc] = rp[order]
        Rb = np.zeros(NBLK, dtype=np.int64)
        for b in range(NBLK):
            Rb[b] = deg_s[b * P:(b + 1) * P].max()
        cores.append(
            dict(lo=lo, nloc=nloc, deg_s=deg_s, node_s=node_s, rp_s=rp_s,
                 es=es, Rb=Rb, slot_of_local=slot_of_local)
        )

    Rb = np.max(np.stack([cc["Rb"] for cc in cores]), axis=0)  # [NBLK]
    assert np.all(np.diff(Rb) >= 0), "Rb must be nondecreasing"
    maxR = int(Rb.max())

    # tile structure (uniform across cores): per round, chunk the block-suffix
    tiles = []  # (r, col0, w, sbase)
    sbase = 0
    for r in range(maxR):
        b_r = int(np.searchsorted(Rb, r + 1))
        col0 = b_r * P
        wtot = (NBLK - b_r) * P
        off = 0
        while off < wtot:
            w = min(TILE_W, wtot - off)
            tiles.append((r, col0 + off, w, sbase + off))
            off += w
        sbase += wtot
    S_E = sbase
    n_sub = S_E // P

    slot_edge = np.full((NCORE, S_E), -1, dtype=np.int64)
    slot_col = np.empty(S_E, dtype=np.int64)
    spos = 0
    for r in range(maxR):
        b_r = int(np.searchsorted(Rb, r + 1))
        cols = np.arange(b_r * P, NBLK * P)
        n_s = len(cols)
        slot_col[spos:spos + n_s] = cols
        for c in range(NCORE):
            cc = cores[c]
            degc = cc["deg_s"][cols]
            rpc = cc["rp_s"][cols]
            has = degc > r
            dup = (~has) & (degc > 0)
            e = np.full(n_s, -1, dtype=np.int64)
            e[has] = cc["es"][rpc[has] + r]
            e[dup] = cc["es"][rpc[dup]]
            slot_edge[c, spos:spos + n_s] = e
        spos += n_s
    assert spos == S_E

    # masked subtiles: any core has a pad slot (-1 edge) on a REAL node there
    sub_masked = np.zeros(n_sub, dtype=bool)
    for si in range(n_sub):
        cols = slot_col[si * P: si * P + P]
        for c in range(NCORE):
            cc = cores[c]
            e = slot_edge[c, si * P: si * P + P]
            real = cc["node_s"][cols] >= 0
            if np.any((e < 0) & real):
                sub_masked[si] = True
                break
    masked_ids = np.where(sub_masked)[0]
    mask_index = {int(s): i for i, s in enumerate(masked_ids)}
    NMASK = max(1, len(masked_ids))

    # ---- split-exchange threshold: blocks < B_star are final once all
    # rounds <= RR are consumed, so their x rows can be exchanged early,
    # overlapped with the remaining rounds' compute ----
    RR = int(os.environ.get("KRR", "5"))
    B_star = int(np.searchsorted(Rb, RR + 2))
    C_star = B_star * P
    TE = next((i for i, t in enumerate(tiles) if t[0] > RR), len(tiles) - 1)

    # request lists: req[c][d] = sorted unique src nodes of core c's edges
    # owned by d, split into early (owner slot < C_star) / late parts
    reqE = [[None] * NCORE for _ in range(NCORE)]
    reqL = [[None] * NCORE for _ in range(NCORE)]
    maxE = maxL = 1
    for c in range(NCORE):
        e = slot_edge[c]
        srcs = np.unique(src[e[e >= 0]])
        bounds = np.searchsorted(srcs, np.arange(1, NCORE) * NODE_LOC)
        parts = np.split(srcs, bounds)
        for dd in range(NCORE):
            lst = parts[dd]
            oslot = cores[dd]["slot_of_local"][lst - dd * NODE_LOC]
            early = oslot < C_star
            reqE[c][dd] = lst[early]
            reqL[c][dd] = lst[~early]
            maxE = max(maxE, len(reqE[c][dd]))
            maxL = max(maxL, len(reqL[c][dd]))
    R_E = ((maxE + P - 1) // P) * P
    R_L = ((maxL + P - 1) // P) * P
    assert NCORE * (R_E + R_L) < 32768, "recvbuf rows must fit int16"

    per_core_inputs = []
    meta = dict(N=N, C=C, D=D, E=E, NODE_LOC=NODE_LOC, NBLK=NBLK,
                S_NODE=S_NODE, S_E=S_E, maxR=maxR, tiles=tiles,
                masked_ids=masked_ids.tolist(), mask_index=mask_index,
                NMASK=NMASK, R_E=R_E, R_L=R_L, n_sub=n_sub,
                B_star=B_star, C_star=C_star, TE=TE, RR=RR)

    def wrappos(n_lst, base, Rpad):
        i = np.arange(n_lst)
        kc = i // GC
        i2 = i % GC
        return base + (i2 % P) * (Rpad // P) + kc * (GC // P) + i2 // P

    for c in range(NCORE):
        cc = cores[c]
        # receive-position map: node -> recvbuf row
        posmap = np.zeros(N, dtype=np.int64)
        for dd in range(NCORE):
            lstE, lstL = reqE[c][dd], reqL[c][dd]
            posmap[lstE] = wrappos(len(lstE), dd * R_E, R_E)
            posmap[lstL] = wrappos(len(lstL), NCORE * R_E + dd * R_L, R_L)
        e = slot_edge[c]
        has_e = e >= 0
        srcn = np.where(has_e, src[np.clip(e, 0, None)], 0)
        srcpos = np.where(has_e, posmap[srcn], 0)
        ncols16 = S_E // 16
        slotidx = np.zeros((16, ncols16), dtype=np.int16)
        base = 0
        while base < S_E:
            n = min(GC, S_E - base)
            w = _wrap16(srcpos[base:base + n], n // 16)
            slotidx[:, base // 16: (base + n) // 16] = w
            base += n
        slotidx_full = np.tile(slotidx, (8, 1))  # [128, S_E/16]

        # send index lists: E segment (cols 0:8*R_E/16) then L segment
        sendidx = np.zeros((16, NCORE * (R_E + R_L) // 16), dtype=np.int16)
        for dd in range(NCORE):
            rows = cc["slot_of_local"][reqE[dd][c] - cc["lo"]]
            rows = np.concatenate([rows, np.zeros(R_E - len(rows), np.int64)])
            sendidx[:, dd * (R_E // 16): (dd + 1) * (R_E // 16)] = \
                _wrap16_chunks(rows, GC)
            off16 = NCORE * R_E // 16
            rows = cc["slot_of_local"][reqL[dd][c] - cc["lo"]]
            rows = np.concatenate([rows, np.zeros(R_L - len(rows), np.int64)])
            sendidx[:, off16 + dd * (R_L // 16): off16 + (dd + 1) * (R_L // 16)] = \
                _wrap16_chunks(rows, GC)
        sendidx_full = np.tile(sendidx, (8, 1))

        # rhs18 for hy (bf16): rows 0:9 = vc[src(e)], rows 9:18 = vc[dst(e)]
        rhs18 = np.zeros((2 * D, S_E), dtype=np.float32)
        dstn = np.where(has_e, dst[np.clip(e, 0, None)], 0)
        rhs18[:D, has_e] = vc[srcn[has_e]].T
        rhs18[D:, has_e] = vc[dstn[has_e]].T

        # mask data [64, NMASK*128]
        mask64 = np.zeros((H, NMASK * P), dtype=np.float32)
        for i, si in enumerate(masked_ids):
            cols = slot_col[si * P: si * P + P]
            ee = slot_edge[c, si * P: si * P + P]
            real = cc["node_s"][cols] >= 0
            dead = (ee < 0) & real
            mask64[:, i * P: (i + 1) * P][:, dead] = NEG

        # node-init features, slot order, transposed
        f36 = np.zeros((feat36.shape[1], S_NODE), dtype=np.float32)
        realn = cc["node_s"] >= 0
        f36[:, realn] = feat36[cc["node_s"][realn]].T

        per_core_inputs.append(dict(
            slotidx=slotidx_full, sendidx=sendidx_full,
            rhs18=rhs18.astype(NPBF16),
            mask64=mask64, feat36T=f36,
        ))

    meta["slot_col"] = slot_col
    meta["cores"] = cores
    return meta, per_core_inputs


def _fold_weights(w):
    """Host-side weight refactoring (see module docstring for the algebra)."""
    out = {}
    f32 = lambda a: np.ascontiguousarray(a, dtype=np.float32)
    bf = lambda a: np.ascontiguousarray(np.asarray(a, dtype=np.float32)).astype(NPBF16)
    D = w["hy_w1"].shape[0] // 3
    b2 = w["fx_b2"].astype(np.float32)          # x = x~ + fx_b2
    yb2 = w["fy_b2"].astype(np.float32)         # y = y~ + fy_b2

    out["hx_w1"] = f32(w["hx_w1"])              # [36, 64]
    out["hx_w2"] = f32(w["hx_w2"])
    out["hx_b1"] = f32(w["hx_b1"][:, None])
    # hx output goes straight into x~ state: subtract fx_b2
    hxb = np.zeros((P, 1), np.float32)
    hxb[H:, 0] = w["hx_b2"] - b2
    out["hxb2"] = hxb                            # [128,1], rows 64:128

    U = w["hy_w1"]
    hyAB = np.vstack([U[2 * D:3 * D] - U[0:D],   # vi = vc[src]
                      U[0:D] + U[D:2 * D]])      # vj = vc[dst]
    hyw = np.zeros((P, P), np.float32)
    hyw[0:2 * D, H:] = hyAB                      # z1 -> out rows 64:128
    hyw[H:, 0:H] = w["hy_w2"]                    # z2 -> out rows 0:64
    out["hyw_comb"] = bf(hyw)                    # [128, 128] bf16
    out["hyw2_hi"] = bf(np.vstack([np.zeros((H, H), np.float32), w["hy_w2"]]))
    hyb = np.zeros((P, 1), np.float32)
    hyb[0:H, 0] = w["hy_b2"] - yb2               # y~ = y - fy_b2
    hyb[H:, 0] = w["hy_b1"]
    out["hyb"] = hyb

    W1 = w["fx_w1"]
    V1 = w["fy_w1"]
    fxA = W1[64:128] + W1[0:64]                  # xj = x[src] (gathered)
    fxB = W1[128:192] - W1[0:64]                 # xi = x[dst] (own)
    fxC = W1[192:256]
    fyB = V1[128:192] - V1[0:64]                 # xj
    fyA = V1[0:64] + V1[64:128]                  # xi
    out["wAB"] = bf(np.vstack([np.hstack([fyB, fxA]),     # K rows 0:64: xj~
                               np.hstack([fyA, fxB])]))   # K rows 64:128: xi~
    out["fxC"] = bf(fxC)                         # [64, 64]
    w2d = np.zeros((P, P), np.float32)
    w2d[0:H, 0:H] = w["fy_w2"]
    w2d[H:, H:] = w["fx_w2"]
    out["w2diag"] = bf(w2d)                      # [128, 128]
    out["fxw2_hi"] = bf(np.vstack([np.zeros((H, H), np.float32), w["fx_w2"]]))
    b1p = np.zeros((P, 1), np.float32)
    b1p[0:H, 0] = w["fy_b1"] + (fyB.T + fyA.T) @ b2
    b1p[H:, 0] = w["fx_b1"] + (fxA.T + fxB.T) @ b2 + fxC.T @ yb2
    out["b1pair"] = b1p                          # [128,1]

    out["feta_w1_hi"] = f32(np.vstack([np.zeros((H, H), np.float32),
                                       w["feta_w1"]]))  # rows 64:128
    out["feta_w2"] = f32(w["feta_w2"])
    out["feta_w3"] = f32(w["feta_w3"])
    out["feta_b1e"] = f32((w["feta_b1"] + w["feta_w1"].T @ b2)[:, None])
    out["feta_b2"] = f32(w["feta_b2"][:, None])
    return out


_WDTYPES = dict(hx_w1=F32, hx_w2=F32, hx_b1=F32, hxb2=F32,
                hyw_comb=BF16, hyw2_hi=BF16, hyb=F32,
                wAB=BF16, fxC=BF16, w2diag=BF16, fxw2_hi=BF16,
                b1pair=F32,
                feta_w1_hi=F32, feta_w2=F32, feta_w3=F32,
                feta_b1e=F32, feta_b2=F32)


def _build(meta, wshapes, LOOP):
    S_NODE, S_E, NBLK = meta["S_NODE"], meta["S_E"], meta["NBLK"]
    NMASK = meta["NMASK"]
    R_E, R_L = meta["R_E"], meta["R_L"]
    B_star, TE = meta["B_star"], meta["TE"]
    C_star = meta["C_star"]
    tiles = meta["tiles"]
    mask_index = meta["mask_index"]
    K18 = 2 * meta["D"]

    nc = bacc.Bacc("TRN2", target_bir_lowering=False, debug=False,
                   num_devices=1 if SIM_SINGLE else NCORE,
                   num_swdge_queues=4)

    # ---- inputs ----
    din = {}
    for name, shp in wshapes.items():
        din[name] = nc.dram_tensor(name, list(shp), _WDTYPES[name],
                                   kind="ExternalInput")
    feat36T = nc.dram_tensor("feat36T", [wshapes["hx_w1"][0], S_NODE], F32,
                             kind="ExternalInput")
    rhs18 = nc.dram_tensor("rhs18", [K18, S_E], BF16, kind="ExternalInput")
    slotidx = nc.dram_tensor("slotidx", [P, S_E // 16], I16, kind="ExternalInput")
    sendidx = nc.dram_tensor("sendidx", [P, NCORE * (R_E + R_L) // 16], I16,
                             kind="ExternalInput")
    mask64 = nc.dram_tensor("mask64", [H, NMASK * P], F32, kind="ExternalInput")

    outslots = nc.dram_tensor("outslots", [S_NODE, 1], F32, kind="ExternalOutput")

    # ---- internal DRAM ----
    yT = nc.dram_tensor("yT", [H, S_E], BF16)
    myslice = nc.dram_tensor("myslice", [S_NODE, P], BF16)
    sendbufE = nc.dram_tensor("sendbufE", [NCORE * R_E, P], BF16)
    sendbufL = nc.dram_tensor("sendbufL", [NCORE * R_L, P], BF16)
    recvbuf = nc.dram_tensor("recvbuf", [NCORE * (R_E + R_L), P], BF16)

    myslice_pview = myslice.ap().rearrange("(b p) f -> p b f", p=P)
    outslots_pview = outslots.ap().rearrange("(b p) o -> p b o", p=P)

    ACT = mybir.ActivationFunctionType
    ALU = mybir.AluOpType

    with tile.TileContext(nc) as tc:
        with (
            tc.tile_pool(name="persist", bufs=1) as pp,
            tc.tile_pool(name="work", bufs=3) as wp,
            tc.tile_pool(name="hpool", bufs=3) as hq,
            tc.tile_pool(name="sendp", bufs=2) as sp,
            tc.tile_pool(name="callbuf", bufs=16) as cbp,
            tc.tile_pool(name="pza", bufs=4, space="PSUM") as pza,
            tc.tile_pool(name="pzb", bufs=2, space="PSUM") as pzb,
            tc.tile_pool(name="ptrp", bufs=2, space="PSUM") as ptrp,
        ):
            # ---- persistent tiles ----
            identb = pp.tile([P, P], BF16, tag="identb")
            make_identity(nc, identb[:])
            W = {}
            for name, shp in wshapes.items():
                t = pp.tile(list(shp), _WDTYPES[name], tag=f"w_{name}")
                nc.sync.dma_start(out=t[:], in_=din[name][:, :])
                W[name] = t
            xt = pp.tile([P, S_NODE], F32, tag="xt")      # rows 64:128 = x~
            xbf = pp.tile([P, S_NODE], BF16, tag="xbf")   # rows 64:128 = bf16(x~)
            staging = pp.tile([P, NBLK * H], BF16, tag="staging")
            staging2 = pp.tile([P, NBLK], F32, tag="staging2")
            sidx = pp.tile([P, S_E // 16], I16, tag="sidx")
            nc.sync.dma_start(out=sidx[:], in_=slotidx[:, :])
            kidx = pp.tile([P, NCORE * (R_E + R_L) // 16], I16, tag="kidx")
            nc.sync.dma_start(out=kidx[:], in_=sendidx[:, :])
            msk = pp.tile([P, NMASK * P], F32, tag="msk")
            nc.sync.dma_start(out=msk[H:P, :], in_=mask64[:, :])

            # zero myslice's pad half once (gathered but never consumed)
            nc.vector.memset(staging[:, :], 0.0)
            nc.sync.dma_start(
                out=myslice_pview[:, :, H:P],
                in_=staging[:].rearrange("p (b f) -> p b f", b=NBLK))

            evac_ct = [0]

            def evac(dst_ap, src_ap):
                if evac_ct[0] % 2 == 0:
                    nc.scalar.copy(out=dst_ap, in_=src_ap)
                else:
                    nc.vector.tensor_copy(out=dst_ap, in_=src_ap)
                evac_ct[0] += 1

            # ---------- x~ scatter-max consume (with mask on flagged subtiles) ----------
            def consume(col0, w, sbase, w2p):
                j = 0
                while j < w // P:
                    gsub = (sbase + j * P) // P
                    if gsub in mask_index:
                        mi = mask_index[gsub]
                        tmp = wp.tile([P, P], F32, tag="mtmp")
                        nc.vector.tensor_tensor(
                            out=tmp[H:P, :], in0=w2p[H:P, j * P:(j + 1) * P],
                            in1=msk[H:P, mi * P:(mi + 1) * P], op=ALU.add)
                        nc.vector.tensor_tensor(
                            out=xt[H:P, col0 + j * P:col0 + (j + 1) * P],
                            in0=xt[H:P, col0 + j * P:col0 + (j + 1) * P],
                            in1=tmp[H:P, :], op=ALU.max)
                        j += 1
                    else:
                        j2 = j
                        while j2 < w // P and ((sbase + j2 * P) // P) not in mask_index:
                            j2 += 1
                        nc.vector.tensor_tensor(
                            out=xt[H:P, col0 + j * P:col0 + j2 * P],
                            in0=xt[H:P, col0 + j * P:col0 + j2 * P],
                            in1=w2p[H:P, j * P:j2 * P], op=ALU.max)
                        j = j2

            # ---------- readback: x~ -> xbf -> myslice (transposed bf16) ----------
            def readback(b0, b1):
                nc.vector.tensor_copy(out=xbf[H:P, b0 * P:b1 * P],
                                      in_=xt[H:P, b0 * P:b1 * P])
                for b in range(b0, b1):
                    ps = ptrp.tile([P, TILE_W], F32, tag="ptr")
                    psb = ps[:].bitcast(BF16)[:, 0:H]
                    nc.tensor.transpose(
                        out=psb, in_=xbf[H:P, b * P:(b + 1) * P],
                        identity=identb[H:P, H:P])
                    evac(staging[:, b * H:(b + 1) * H], psb)
                nc.sync.dma_start(
                    out=myslice_pview[:, b0:b1, 0:H],
                    in_=staging[:, b0 * H:b1 * H].rearrange(
                        "p (b f) -> p b f", b=b1 - b0))

            # queue_num must track the global Pool-DMA instruction order:
            # tile_sem_assignment rotates DMASW sem lanes per instruction and
            # each lane is serviced by the matching SWDGE queue.
            gq = [0]

            def next_q():
                q = gq[0] % 4
                gq[0] += 1
                return q

            # ---------- exchange halves: myslice -> sendbuf -> A2A -> recvbuf --
            # seg 'E': rows with owner slot < C_star (final after round RR);
            # seg 'L': the rest. DMAs issued from the gpsimd queue so the sync
            # queue stays free for compute-side loads.
            def exchange(seg):
                Rp = R_E if seg == "E" else R_L
                kbase = 0 if seg == "E" else NCORE * R_E
                sbuf = sendbufE if seg == "E" else sendbufL
                src_ap = myslice[0:C_star, :] if seg == "E" else myslice[:, :]
                for dd in range(NCORE):
                    st = sp.tile([P, max(R_E, R_L) // P, P], BF16, tag="sendt")
                    off = 0
                    while off < Rp:
                        n = min(GC, Rp - off)
                        nc.gpsimd.dma_gather(
                            out_ap=st[:, off // P:(off + n) // P, :],
                            in_ap=src_ap,
                            idxs_ap=kidx[:, (kbase + dd * Rp + off) // 16:
                                         (kbase + dd * Rp + off + n) // 16],
                            num_idxs=n, num_idxs_reg=n, elem_size=P,
                            queue_num=next_q())
                        off += n
                    dv = sbuf.ap()[dd * Rp:(dd + 1) * Rp, :].rearrange(
                        "(p j) f -> p (j f)", p=P)
                    nc.sync.dma_start(
                        out=dv,
                        in_=st[:, 0:Rp // P, :].rearrange("p j f -> p (j f)"))
                rv = (recvbuf.ap()[0:NCORE * R_E, :] if seg == "E"
                      else recvbuf.ap()[NCORE * R_E:, :])
                if SIM_SINGLE:
                    nc.sync.dma_start(
                        out=rv.rearrange("(p a) f -> p (a f)", p=P),
                        in_=sbuf.ap().rearrange("(p a) f -> p (a f)", p=P))
                else:
                    nc.gpsimd.collective_compute(
                        "AllToAll", ALU.bypass,
                        replica_groups=[list(range(NCORE))],
                        ins=[sbuf.ap().bitcast(F32)],
                        outs=[rv.bitcast(F32)])

            # ---------- slot gather: recvbuf -> callbuf chunks (row-major) ----------
            def slot_gather():
                bufs = []
                base = 0
                while base < S_E:
                    n = min(GC, S_E - base)
                    st = cbp.tile([P, GC // P, P], BF16, tag="cb")
                    nc.gpsimd.dma_gather(
                        out_ap=st[:, : n // P, :], in_ap=recvbuf[:, :],
                        idxs_ap=sidx[:, base // 16:(base + n) // 16],
                        num_idxs=n, num_idxs_reg=n, elem_size=P,
                        queue_num=next_q())
                    bufs.append(st)
                    base += n
                return bufs

            # ---------- per-tile stacked rhs: rows 0:64 = xj~^T (PE-transposed
            # gathered blocks), rows 64:128 = xi~ (aligned copy of xbf) ----------
            def make_stk(bufs, col0, sbase, w):
                stk = wp.tile([P, TILE_W], BF16, tag="stk")
                for j in range(w // P):
                    s = sbase + j * P
                    g = bufs[s // GC][:, (s % GC) // P, 0:H]
                    ps = ptrp.tile([P, TILE_W], F32, tag="ptr")
                    psb = ps[:].bitcast(BF16)[0:H, 0:P]
                    nc.tensor.transpose(out=psb, in_=g, identity=identb[:, :])
                    evac(stk[0:H, j * P:(j + 1) * P], psb)
                nc.sync.dma_start(out=stk[H:P, :w],
                                  in_=xbf[H:P, col0:col0 + w])
                return stk

            # ---------- fused phase: fy(k) then fx(k), sharing gathered x ----------
            def fused_phase(k, bufs, do_exchange):
                KSUB = int(os.environ.get("KSUB", "9"))
                with_fy = k > 0
                write_y = k < LOOP - 1
                hp_cur = None     # [h1y(t); h1x(t-1)]
                pend = None       # (col0, w, sbase) of tile t-1 awaiting z2x
                early_done = False
                for ti, (r, col0, w, sbase) in enumerate(tiles):
                    if ti >= int(os.environ.get("KTILES", "9999")):
                        break
                    # once blocks < B_star are final, transpose their x rows
                    # to myslice while the remaining rounds compute; the E
                    # send-gathers (emitted after the loop) then run on the
                    # Pool queue during the phase tail
                    if do_exchange and not early_done and ti > TE:
                        readback(0, B_star)
                        early_done = True
                    stk = make_stk(bufs, col0, sbase, w)
                    if KSUB < 1:
                        continue
                    z = pza.tile([P, TILE_W], F32, tag="z")
                    nc.tensor.matmul(z[:, :w], W["wAB"][:], stk[:, :w],
                                     start=True, stop=True)
                    if KSUB < 2:
                        continue
                    yt = wp.tile([H, TILE_W], BF16, tag="yt")
                    nc.sync.dma_start(out=yt[:, :w], in_=yT[:, sbase:sbase + w])
                    if with_fy:
                        if hp_cur is None:
                            hp_cur = hq.tile([P, TILE_W], BF16, tag="hp")
                            nc.vector.memset(hp_cur[H:P, :], 0.0)
                        nc.scalar.activation(out=hp_cur[0:H, :w], in_=z[0:H, :w],
                                             func=ACT.Relu,
                                             bias=W["b1pair"][0:H, 0:1])
                        wz = max(w, pend[1]) if pend is not None else w
                        if pend is not None and pend[1] > w:
                            # h1y(t) gap: z2 streams wz cols, relu wrote only w
                            nc.vector.memset(hp_cur[0:H, w:pend[1]], 0.0)
                        if pend is not None and w > pend[1]:
                            # h1x(t-1) gap: written only to pend[1]
                            nc.vector.memset(hp_cur[H:P, pend[1]:w], 0.0)
                        w2p = pzb.tile([P, TILE_W], F32, tag="w2p")
                        nc.tensor.matmul(w2p[:, :wz], W["w2diag"][:],
                                         hp_cur[:, :wz], start=True, stop=True)
                        # y~ = max(y~, z2y(t))
                        nc.vector.tensor_tensor(out=yt[:, :w], in0=yt[:, :w],
                                                in1=w2p[0:H, :w], op=ALU.max)
                        if write_y:
                            nc.sync.dma_start(out=yT[:, sbase:sbase + w],
                                              in_=yt[:, :w])
                        if pend is not None:
                            consume(pend[0], pend[1], pend[2], w2p)
                        pend = (col0, w, sbase)
                    nc.tensor.matmul(z[H:P, :w], W["fxC"][:], yt[:, :w],
                                     start=False, stop=True, skip_group_check=True)
                    if KSUB < 3:
                        continue
                    hp_next = hq.tile([P, TILE_W], BF16, tag="hp")
                    nc.scalar.activation(out=hp_next[H:P, :w], in_=z[H:P, :w],
                                         func=ACT.Relu, bias=W["b1pair"][H:P, 0:1])
                    if KSUB < 4:
                        continue
                    if not with_fy:
                        w2p = pzb.tile([P, TILE_W], F32, tag="w2p")
                        nc.tensor.matmul(w2p[H:P, :w], W["fxw2_hi"][H:P, :],
                                         hp_next[H:P, :w], start=True, stop=True)
                        consume(col0, w, sbase, w2p)
                    hp_cur = hp_next
                if with_fy:
                    # flush: z2x of the last tile
                    lc, lw, lsb = pend
                    w2p = pzb.tile([P, TILE_W], F32, tag="w2p")
                    nc.tensor.matmul(w2p[H:P, :lw], W["fxw2_hi"][H:P, :],
                                     hp_cur[H:P, :lw], start=True, stop=True)
                    consume(lc, lw, lsb, w2p)
                if do_exchange:
                    if not early_done:
                        readback(0, B_star)
                    exchange("E")
                    readback(B_star, NBLK)
                    exchange("L")

            # ---------- init: hx (f32) ----------
            K36 = wshapes["hx_w1"][0]
            off = 0
            while off < S_NODE:
                w = min(TILE_W, S_NODE - off)
                ft = wp.tile([K36, TILE_W], F32, tag="ft")
                nc.sync.dma_start(out=ft[:, :w], in_=feat36T[:, off:off + w])
                z1 = pza.tile([P, TILE_W], F32, tag="z")
                nc.tensor.matmul(z1[0:H, :w], W["hx_w1"][:], ft[:, :w],
                                 start=True, stop=True)
                h1 = wp.tile([H, TILE_W], F32, tag="h1")
                nc.scalar.activation(out=h1[:, :w], in_=z1[0:H, :w],
                                     func=ACT.Relu, bias=W["hx_b1"][:, 0:1])
                z2 = pza.tile([P, TILE_W], F32, tag="z")
                nc.tensor.matmul(z2[H:P, :w], W["hx_w2"][:], h1[:, :w],
                                 start=True, stop=True)
                nc.scalar.activation(out=xt[H:P, off:off + w], in_=z2[H:P, :w],
                                     func=ACT.Identity, bias=W["hxb2"][H:P, 0:1])
                off += w

            # ---------- initial exchange of x0 (issued before hy init so the
            # send-gathers + AllToAll overlap hy's PE/ACT work) ----------
            KSTAGE = int(os.environ.get("KSTAGE", "0"))
            readback(0, NBLK)
            exchange("E")
            exchange("L")

            # ---------- init: hy (bf16, one pipelined pass per tile) ----------
            # pass t: z1(t) (rows 64:128) from rt[0:18]=r18(t);
            #         z2(t-1) (rows 0:64) from rt[64:128]=h1y(t-1)
            rt_cur = wp.tile([P, TILE_W], BF16, tag="rt")
            nc.vector.memset(rt_cur[:, :], 0.0)
            nc.sync.dma_start(out=rt_cur[0:K18, :tiles[0][2]],
                              in_=rhs18[:, 0:tiles[0][2]])
            pw = 0
            psb_prev = 0
            for ti, (r, col0, w, sbase) in enumerate(tiles):
                wz = max(w, pw)
                zi = pza.tile([P, TILE_W], F32, tag="z")
                nc.tensor.matmul(zi[:, :wz], W["hyw_comb"][:], rt_cur[:, :wz],
                                 start=True, stop=True)
                if ti > 0:
                    yt0 = wp.tile([H, TILE_W], BF16, tag="yt")
                    nc.scalar.activation(out=yt0[:, :pw], in_=zi[0:H, :pw],
                                         func=ACT.Identity, bias=W["hyb"][0:H, 0:1])
                    nc.sync.dma_start(out=yT[:, psb_prev:psb_prev + pw],
                                      in_=yt0[:, :pw])
                if ti + 1 < len(tiles):
                    nw = tiles[ti + 1][2]
                    rt_next = wp.tile([P, TILE_W], BF16, tag="rt")
                    nc.vector.memset(rt_next[:, :], 0.0)
                    nc.sync.dma_start(
                        out=rt_next[0:K18, :nw],
                        in_=rhs18[:, tiles[ti + 1][3]:tiles[ti + 1][3] + nw])
                    nc.scalar.activation(out=rt_next[H:P, :w], in_=zi[H:P, :w],
                                         func=ACT.Relu, bias=W["hyb"][H:P, 0:1])
                    rt_cur = rt_next
                else:
                    # flush: z2 of the last tile via hy_w2-only pass
                    hlast = wp.tile([P, TILE_W], BF16, tag="rt")
                    nc.scalar.activation(out=hlast[H:P, :w], in_=zi[H:P, :w],
                                         func=ACT.Relu, bias=W["hyb"][H:P, 0:1])
                    zf = pza.tile([P, TILE_W], F32, tag="z")
                    nc.tensor.matmul(zf[0:H, :w], W["hyw2_hi"][H:P, :],
                                     hlast[H:P, :w], start=True, stop=True)
                    ytf = wp.tile([H, TILE_W], BF16, tag="yt")
                    nc.scalar.activation(out=ytf[:, :w], in_=zf[0:H, :w],
                                         func=ACT.Identity, bias=W["hyb"][0:H, 0:1])
                    nc.sync.dma_start(out=yT[:, sbase:sbase + w], in_=ytf[:, :w])
                pw = w
                psb_prev = sbase

            # ---------- iterations ----------
            if KSTAGE != 1:
                for k in range(LOOP):
                    bufs = slot_gather()
                    if KSTAGE == 2:
                        break
                    fused_phase(k, bufs, do_exchange=(k < LOOP - 1))
                    if KSTAGE == 3:
                        break

            # ---------- final MLP (f32) ----------
            off = 0
            while off < S_NODE:
                w = min(TILE_W, S_NODE - off)
                z1 = pza.tile([P, TILE_W], F32, tag="z")
                nc.tensor.matmul(z1[0:H, :w], W["feta_w1_hi"][H:P, :],
                                 xt[H:P, off:off + w], start=True, stop=True)
                h1 = wp.tile([H, TILE_W], F32, tag="h1")
                nc.scalar.activation(out=h1[:, :w], in_=z1[0:H, :w],
                                     func=ACT.Relu, bias=W["feta_b1e"][:, 0:1])
                z2 = pza.tile([P, TILE_W], F32, tag="z")
                nc.tensor.matmul(z2[0:H, :w], W["feta_w2"][:], h1[:, :w],
                                 start=True, stop=True)
                h2 = wp.tile([H, TILE_W], F32, tag="h2")
                nc.scalar.activation(out=h2[:, :w], in_=z2[0:H, :w],
                                     func=ACT.Relu, bias=W["feta_b2"][:, 0:1])
                for j in range(w // P):
                    b = (off + j * P) // P
                    ps = pzb.tile([P, TILE_W], F32, tag="w2p")
                    nc.tensor.matmul(ps[:, 0:1], h2[:, j * P:(j + 1) * P],
                                     W["feta_w3"][:], start=True, stop=True)
                    evac(staging2[:, b:b + 1], ps[:, 0:1])
                off += w
            nc.sync.dma_start(
                out=outslots_pview,
                in_=staging2[:].rearrange("p (b o) -> p b o", b=NBLK))

    _log(f"built program: {S_E=} {len(tiles)=} masks={NMASK} "
         f"R_E={R_E} R_L={R_L} B*={B_star} TE={TE}")
    nc.compile()
    _log("compiled")
    return nc


def kernel(**inputs):
    global LAST_EXEC_NS, LAST_TRACE
    v = np.asarray(inputs["v"], dtype=np.float32)
    labels = np.asarray(inputs["labels"], dtype=np.float32)
    edge_index = np.asarray(inputs["edge_index"]).astype(np.int64)
    LOOP = int(np.asarray(inputs["loop"]))

    import hashlib
    ck = hashlib.sha1(edge_index.tobytes()).hexdigest() + f"_{LOOP}_{v.shape}"
    if ck in _BUILD_CACHE:
        meta, pci, nc = _BUILD_CACHE[ck]
    else:
        meta, pci, nc = None, None, None
    if meta is None:
        meta, pci = _preprocess(v, labels, edge_index)
    wf = _fold_weights({k: np.asarray(val, dtype=np.float32)
                        for k, val in inputs.items()
                        if k not in ("v", "labels", "edge_index", "loop")})
    wnames = list(_WDTYPES.keys())
    wshapes = {n: wf[n].shape for n in wnames}

    if nc is None:
        nc = _build(meta, wshapes, LOOP)
        _BUILD_CACHE[ck] = (meta, pci, nc)

    in_maps = []
    for c in range(NCORE):
        m = {n: wf[n] for n in wnames}
        m["feat36T"] = pci[c]["feat36T"]
        m["rhs18"] = pci[c]["rhs18"]
        m["slotidx"] = pci[c]["slotidx"]
        m["sendidx"] = pci[c]["sendidx"]
        m["mask64"] = pci[c]["mask64"]
        in_maps.append(m)

    res = run_bass_kernel_spmd(nc, in_maps, core_ids=list(range(NCORE)),
                               tmpdir=os.environ.get("BASS_TMPDIR") or None)
    LAST_EXEC_NS = res.exec_time_ns
    LAST_TRACE = res.instructions_and_trace

    N = meta["N"]
    out = np.zeros((N, 1), dtype=np.float32)
    for c in range(NCORE):
        cc = meta["cores"][c]
        slots = cc["slot_of_local"]
        vals = res.results[c]["outslots"][:, 0]
        out[cc["lo"]:cc["lo"] + cc["nloc"], 0] = vals[slots]
    return out
